# revision 12
# baseline (speedup 1.0000x reference)
"""Causal multi-head attention on 8 TRN2 NeuronCores, data-parallel over batch.

Per-core work (batch=1): q/k/v projections, per-head causal softmax
attention. Two-pass score computation keeps every engine on its fast path:

  pass 1: S = (Q K^T)/8 + causal mask  (PE, f32 PSUM) -> row max m  (DVE)
  pass 2: S'^T = K Q^T - m + mask      (PE) -> exp on Scalar, which writes
          P^T straight to SBUF f16 -- the exp IS the PSUM evacuation, so no
          PE transposes and no separate PSUM->SBUF copies are needed.

The "- m" ride-along: Q^T tiles are stored per head in QTE with 8 spare
partition rows; each q-tile's negated row maxes (all 8 heads, f16) are
transposed once on the PE and broadcast-written into those rows. K^T tiles
in KTE carry a constant one-hot indicator in the matching rows, so the
K=72 contraction adds exactly -m[h] per output column. Softmax denominators
come from a ones-column appended to V (65-wide AV matmuls); the final
normalize runs on GpSimd from an SBUF copy of the AV accumulators.
Host-side prep: inputs transposed to [D_IN, L] and cast to fp16.
"""

import sys

sys.path.insert(0, "/opt/trn_rl_repo")

import numpy as np

import concourse.bacc as bacc
import concourse.tile as tile
from concourse import mybir
from concourse.bass_utils import run_bass_kernel_spmd
from concourse.masks import make_identity

B, L, DIN, H, D = 8, 1024, 512, 8, 64
HD = H * D
F32 = mybir.dt.float32
F16 = mybir.dt.float16
N_CORES = 8
MASK_VAL = -60000.0

_cached = {}


def _build():
    nc = bacc.Bacc("TRN2", target_bir_lowering=False, debug=False,
                   enable_asserts=False, num_devices=N_CORES)

    qt_d = nc.dram_tensor("qt", [DIN, L], F16, kind="ExternalInput").ap()
    kt_d = nc.dram_tensor("kt", [DIN, L], F16, kind="ExternalInput").ap()
    vt_d = nc.dram_tensor("vt", [DIN, L], F16, kind="ExternalInput").ap()
    wq_d = nc.dram_tensor("wq", [DIN, HD], F16, kind="ExternalInput").ap()
    wk_d = nc.dram_tensor("wk", [DIN, HD], F16, kind="ExternalInput").ap()
    wv_d = nc.dram_tensor("wv", [DIN, HD], F16, kind="ExternalInput").ap()
    out_d = nc.dram_tensor("out", [L, HD], F32, kind="ExternalOutput").ap()

    with tile.TileContext(nc) as tc:
        _body(tc, out_d, qt_d, kt_d, vt_d, wq_d, wk_d, wv_d)
    nc.compile()
    return nc


def _body(tc, out_d, qt_d, kt_d, vt_d, wq_d, wk_d, wv_d):
    nc = tc.nc
    from contextlib import ExitStack
    with ExitStack() as ctx:
        const = ctx.enter_context(tc.tile_pool(name="const", bufs=1))
        big = ctx.enter_context(tc.tile_pool(name="big", bufs=1))
        sb = ctx.enter_context(tc.tile_pool(name="sb", bufs=6))
        # PSUM: 3 rotating S/S'T tiles (f32, 2 banks each) + av_a + av_b = 8
        ps_s = ctx.enter_context(tc.tile_pool(name="pss", bufs=3, space="PSUM"))
        ps_av = ctx.enter_context(tc.tile_pool(name="psav", bufs=1, space="PSUM"))

        ident = const.tile([128, 128], F16)
        make_identity(nc, ident[:])
        # cmaskT adds MASK_VAL where k > q when used as lhsT against a [q, k]
        # tile; cmask2 is its transpose, for the [k, q]-layout pass-2 tiles.
        cmaskT = const.tile([128, 128], F16)
        nc.gpsimd.memset(cmaskT[:], MASK_VAL)
        nc.gpsimd.affine_select(
            out=cmaskT[:], in_=cmaskT[:],
            compare_op=mybir.AluOpType.is_gt, fill=0.0,
            base=0, pattern=[[-1, 128]], channel_multiplier=1)
        cmask2 = const.tile([128, 128], F16)
        cm_ps = ps_s.tile([128, 128], F16, tag="S", name="cmps")
        nc.tensor.transpose(cm_ps[:], cmaskT[:], ident[:])
        nc.vector.tensor_copy(cmask2[:], cm_ps[:])

        # ---- load inputs (transposed, fp16), one DMA per DIN-chunk piece
        xq = big.tile([128, 4, L], F16)
        xk = big.tile([128, 4, L], F16)
        xv = big.tile([128, 4, L], F16)
        wq = big.tile([128, 4, HD], F16)
        wk = big.tile([128, 4, HD], F16)
        wv = big.tile([128, 4, HD], F16)
        for t, d in ((wq, wq_d), (xq, qt_d), (wk, wk_d), (xk, kt_d),
                     (wv, wv_d), (xv, vt_d)):
            r = d.rearrange("(c p) l -> p c l", p=128)
            for c in range(4):
                nc.sync.dma_start(t[:, c, :], r[:, c, :])

        # QTE/KTE: per-head qT/kT in rows 0-63; rows 64-71 carry the -max
        # ride-along (QTE: per-q-tile negated maxes; KTE: constant one-hot).
        QTE = big.tile([128, 8, L], F16)
        KTE = big.tile([128, 8, L], F16)
        # spare rows zeroed so both passes can run full K=128 contraction
        # (zero rows add nothing); KTE rows 64-71 get the one-hot indicator
        # KTE[64+j, h, :] = (j == h) from the 8x8 identity block of `ident`.
        # All three fills go through broadcast DMAs to keep the DVE/Scalar
        # queues free during the load window.
        zstrip = big.tile([128, L], F16)
        nc.vector.memset(zstrip[:], 0.0)
        ostrip = big.tile([128, L], F16)
        nc.gpsimd.memset(ostrip[:], 1.0)
        for j in range(8):
            nc.sync.dma_start(QTE[64:128, j, :], zstrip[64:128, :])
            nc.sync.dma_start(KTE[64:128, j, :], zstrip[64:128, :])
        for j in range(8):
            nc.sync.dma_start(KTE[64 + j:65 + j, j, :], ostrip[64 + j:65 + j, :])

        # v65: V rows with a ones column appended per head; the attention*V
        # matmul's 65th output column is then the softmax denominator.
        v65 = big.tile([128, 8, 8, 65], F16)
        nc.gpsimd.memset(v65[:, :, :, 64:65], 1.0)

        # PE warm-up: dummy matmuls while the loads stream in, so the
        # clock governor sees sustained activity before projections.
        warm = const.tile([128, 512], F16)
        nc.vector.memset(warm[:], 0.0)
        wps = ps_s.tile([128, 1024], F32, tag="S")
        for i in range(16):
            nc.tensor.matmul(wps[:, 0:512], lhsT=warm[:, 0:128], rhs=warm[:],
                             start=(i == 0), stop=(i == 15))

        # ---- projections (fp16 matmuls, f32 psum)
        for w_sb, x_sb, dst in ((wq, xq, QTE), (wk, xk, KTE)):
            for t in range(4):
                for s in range(2):
                    pp = ps_s.tile([128, 512], F32, tag="S")
                    for c in range(4):
                        nc.tensor.matmul(
                            pp[:],
                            lhsT=w_sb[:, c, t * 128:(t + 1) * 128],
                            rhs=x_sb[:, c, s * 512:(s + 1) * 512],
                            start=(c == 0), stop=(c == 3))
                    sl = slice(s * 512, (s + 1) * 512)
                    if dst is QTE:
                        nc.scalar.copy(dst[0:64, 2 * t, sl], pp[0:64, :])
                        nc.scalar.copy(dst[0:64, 2 * t + 1, sl], pp[64:128, :])
                    else:
                        nc.vector.tensor_copy(dst[0:64, 2 * t, sl], pp[0:64, :])
                        nc.vector.tensor_copy(dst[0:64, 2 * t + 1, sl],
                                              pp[64:128, :])

        def emit_vproj():
            for lt in range(8):
                pp = ps_s.tile([128, 512], F32, name=f"ppv{lt}", tag="S")
                for c in range(4):
                    nc.tensor.matmul(
                        pp[:],
                        lhsT=xv[:, c, lt * 128:(lt + 1) * 128],
                        rhs=wv[:, c, :],
                        start=(c == 0), stop=(c == 3))
                dstv = v65[:, lt, :, 0:64]
                srcv = pp[:].rearrange("p (h d) -> p h d", h=8)
                if lt % 2 == 0:
                    nc.vector.tensor_copy(dstv, srcv)
                else:
                    nc.scalar.copy(dstv, srcv)

        # ---- attention, software-pipelined across q-tiles:
        # pass1(qt): per head: S = QK^T + mask (PE) -> row max (DVE, f16)
        # bridge(qt): nm [128,8] --PE transpose--> [8,128] -> broadcast into
        #             QTE rows 64-71 at this q-tile's columns
        # pass2(qt): per head: S'^T = K Q^T - m + mask (PE) -> exp (Scalar,
        #            writes P^T to SBUF) -> AV matmuls (65 cols)
        # Emission interleaves pass1(qt-1) with pass2(qt) so the PE never
        # waits on the per-q-tile max barrier.
        def emit_pass1(qt):
            Lq0, Lk = qt * 128, (qt + 1) * 128
            nm = sb.tile([128, 8], F16, name=f"nm{qt}", tag="nm", bufs=3)
            for h in range(8):
                S = ps_s.tile([128, 1024], F32, name=f"S{qt}_{h}", tag="S")
                for w in range(0, Lk, 512):
                    n = min(512, Lk - w)
                    diag = (w + n == Lk)
                    nc.tensor.matmul(
                        S[:, w:w + n],
                        lhsT=QTE[0:128, h, Lq0:Lq0 + 128],
                        rhs=KTE[0:128, h, w:w + n],
                        start=True, stop=not diag)
                    if diag:
                        nc.tensor.matmul(S[:, Lk - 128:Lk], lhsT=cmaskT[:],
                                         rhs=ident[:], start=False, stop=True)
                nc.vector.reduce_max(nm[:, h:h + 1], S[:, :Lk],
                                     axis=mybir.AxisListType.X, negate=True)
            return nm

        def emit_bridge(qt, nm):
            Lq0 = qt * 128
            nmT = ps_s.tile([8, 128], F16, name=f"nmT{qt}", tag="S")
            nc.tensor.transpose(nmT[:], nm[:], ident[:])
            nc.vector.tensor_copy(
                QTE[64:72, :, Lq0:Lq0 + 128],
                nmT[:].unsqueeze(1).broadcast_to([8, 8, 128]))

        def emit_st_exp(qt, h):
            Lq0, Lk, nkc = qt * 128, (qt + 1) * 128, qt + 1
            ST = ps_s.tile([128, 1024], F32, name=f"ST{qt}_{h}", tag="S")
            for kc in range(nkc):
                diag = (kc == qt)
                nc.tensor.matmul(
                    ST[:, kc * 128:(kc + 1) * 128],
                    lhsT=KTE[0:128, h, kc * 128:(kc + 1) * 128],
                    rhs=QTE[0:128, h, Lq0:Lq0 + 128],
                    start=True, stop=not diag)
                if diag:
                    nc.tensor.matmul(ST[:, kc * 128:(kc + 1) * 128],
                                     lhsT=cmask2[:], rhs=ident[:],
                                     start=False, stop=True)
            pTs = sb.tile([128, 8, 128], F16, name=f"pTs{qt}_{h}",
                          tag="pTs", bufs=4)
            pTf = pTs[:].rearrange("p c q -> p (c q)")
            nc.scalar.activation(pTf[:, :Lk], ST[:, :Lk],
                                 mybir.ActivationFunctionType.Exp)
            return pTs

        def emit_av(qt, h, pTs, avs):
            av = avs[h // 4]
            for kc in range(qt + 1):
                nc.tensor.matmul(
                    av[:, h % 4, :],
                    lhsT=pTs[:, kc, :],
                    rhs=v65[:, kc, h, :],
                    start=(kc == 0), stop=(kc == qt))

        def emit_finish(qt, avs):
            Lq0 = qt * 128
            av_a, av_b = avs
            avsb = sb.tile([128, 8, 65], F32, name=f"avsb{qt}", tag="avsb",
                           bufs=2)
            nc.scalar.copy(avsb[:, 0:4, :], av_a[:])
            nc.scalar.copy(avsb[:, 4:8, :], av_b[:])
            rec = sb.tile([128, 8], F32, tag="rec")
            nc.vector.reciprocal(rec[:], avsb[:, :, 64])
            out_sb = sb.tile([128, HD], F32, tag="osb")
            nc.gpsimd.tensor_mul(
                out_sb[:].rearrange("p (h d) -> p h d", h=8),
                avsb[:, :, 0:64],
                rec[:].unsqueeze(2).broadcast_to([128, 8, 64]))
            nc.sync.dma_start(out_d[Lq0:Lq0 + 128, :], out_sb[:])

        def emit_s1_head(qt, h, nm):
            Lq0, Lk = qt * 128, (qt + 1) * 128
            S = ps_s.tile([128, 1024], F32, name=f"S{qt}_{h}", tag="S")
            for w in range(0, Lk, 512):
                n = min(512, Lk - w)
                diag = (w + n == Lk)
                nc.tensor.matmul(
                    S[:, w:w + n],
                    lhsT=QTE[0:128, h, Lq0:Lq0 + 128],
                    rhs=KTE[0:128, h, w:w + n],
                    start=True, stop=not diag)
                if diag:
                    nc.tensor.matmul(S[:, Lk - 128:Lk], lhsT=cmaskT[:],
                                     rhs=ident[:], start=False, stop=True)
            nc.vector.reduce_max(nm[:, h:h + 1], S[:, :Lk],
                                 axis=mybir.AxisListType.X, negate=True)

        nm = emit_pass1(7)
        emit_vproj()  # PE work covering the first q-tile's max latency
        emit_bridge(7, nm)
        for qt in range(7, -1, -1):
            av_a = ps_av.tile([128, 4, 65], F32, name=f"ava{qt}", tag="av_a")
            av_b = ps_av.tile([128, 4, 65], F32, name=f"avb{qt}", tag="av_b")
            avs = (av_a, av_b)
            if qt > 0:
                nm = sb.tile([128, 8], F16, name=f"nm{qt - 1}", tag="nm",
                             bufs=3)
            pTss = []
            for h in range(8):
                pTss.append(emit_st_exp(qt, h))
                # next q-tile's pass 1 runs in the first half so its maxes
                # are ready for the bridge at h == 5
                if qt > 0 and h < 4:
                    emit_s1_head(qt - 1, 2 * h, nm)
                    emit_s1_head(qt - 1, 2 * h + 1, nm)
                if qt > 0 and h == 5:
                    emit_bridge(qt - 1, nm)
                if h >= 1:
                    emit_av(qt, h - 1, pTss[h - 1], avs)
            emit_av(qt, 7, pTss[7], avs)
            emit_finish(qt, avs)


def kernel(Q_seq, K_seq, V_seq, WQ, WK, WV, _trace=False):
    if "nc" not in _cached:
        _cached["nc"] = _build()
    nc = _cached["nc"]

    wq16 = (np.asarray(WQ, dtype=np.float32) * 0.125).astype(np.float16)
    wk16 = np.asarray(WK, dtype=np.float16)
    wv16 = np.asarray(WV, dtype=np.float16)
    in_maps = []
    for b in range(N_CORES):
        in_maps.append({
            "qt": np.ascontiguousarray(np.asarray(Q_seq[b]).T.astype(np.float16)),
            "kt": np.ascontiguousarray(np.asarray(K_seq[b]).T.astype(np.float16)),
            "vt": np.ascontiguousarray(np.asarray(V_seq[b]).T.astype(np.float16)),
            "wq": wq16, "wk": wk16, "wv": wv16,
        })
    res = run_bass_kernel_spmd(nc, in_maps, core_ids=list(range(N_CORES)),
                               trace=_trace)
    out = np.stack([res.results[b]["out"] for b in range(N_CORES)], axis=0)
    if _trace:
        kernel.last_exec_time_ns = res.exec_time_ns
        kernel.last_results = res
    return out


# revision 14
# speedup vs baseline: 1.1928x; 1.1928x over previous
"""Causal multi-head attention on 8 TRN2 NeuronCores, data-parallel over batch.

Per-core work (batch=1): q/k/v projections, per-head causal softmax
attention. Two-pass score computation keeps every engine on its fast path:

  pass 1: S = (Q K^T)/8 + causal mask  (PE, f32 PSUM) -> row max m  (DVE)
  pass 2: S'^T = K Q^T - m + mask      (PE) -> exp on Scalar, which writes
          P^T straight to SBUF f16 -- the exp IS the PSUM evacuation, so no
          PE transposes and no separate PSUM->SBUF copies are needed.

The "- m" ride-along: Q^T tiles are stored per head in QTE with 8 spare
partition rows; each q-tile's negated row maxes (all 8 heads, f16) are
transposed once on the PE and broadcast-written into those rows. K^T tiles
in KTE carry a constant one-hot indicator in the matching rows, so the
K=72 contraction adds exactly -m[h] per output column. Softmax denominators
come from a ones-column appended to V (65-wide AV matmuls); the final
normalize runs on GpSimd from an SBUF copy of the AV accumulators.
Host-side prep: inputs transposed to [D_IN, L] and cast to fp16.
"""

import sys

sys.path.insert(0, "/opt/trn_rl_repo")

import numpy as np

import concourse.bacc as bacc
import concourse.tile as tile
from concourse import mybir
from concourse.bass_utils import run_bass_kernel_spmd
from concourse.masks import make_identity

B, L, DIN, H, D = 8, 1024, 512, 8, 64
HD = H * D
F32 = mybir.dt.float32
F16 = mybir.dt.float16
N_CORES = 8
MASK_VAL = -60000.0

_cached = {}


def _build():
    nc = bacc.Bacc("TRN2", target_bir_lowering=False, debug=False,
                   enable_asserts=False, num_devices=N_CORES)

    qt_d = nc.dram_tensor("qt", [DIN, L], F16, kind="ExternalInput").ap()
    kt_d = nc.dram_tensor("kt", [DIN, L], F16, kind="ExternalInput").ap()
    vt_d = nc.dram_tensor("vt", [DIN, L], F16, kind="ExternalInput").ap()
    wq_d = nc.dram_tensor("wq", [DIN, HD], F16, kind="ExternalInput").ap()
    wk_d = nc.dram_tensor("wk", [DIN, HD], F16, kind="ExternalInput").ap()
    wv_d = nc.dram_tensor("wv", [DIN, HD], F16, kind="ExternalInput").ap()
    out_d = nc.dram_tensor("out", [L, HD], F32, kind="ExternalOutput").ap()

    with tile.TileContext(nc) as tc:
        _body(tc, out_d, qt_d, kt_d, vt_d, wq_d, wk_d, wv_d)
    nc.compile()
    return nc


def _body(tc, out_d, qt_d, kt_d, vt_d, wq_d, wk_d, wv_d):
    nc = tc.nc
    from contextlib import ExitStack
    with ExitStack() as ctx:
        const = ctx.enter_context(tc.tile_pool(name="const", bufs=1))
        big = ctx.enter_context(tc.tile_pool(name="big", bufs=1))
        sb = ctx.enter_context(tc.tile_pool(name="sb", bufs=6))
        # PSUM: 3 rotating S/S'T tiles (f32, 2 banks each) + av_a + av_b = 8
        ps_s = ctx.enter_context(tc.tile_pool(name="pss", bufs=3, space="PSUM"))
        ps_av = ctx.enter_context(tc.tile_pool(name="psav", bufs=1, space="PSUM"))

        ident = const.tile([128, 128], F16)
        make_identity(nc, ident[:])
        # cmaskT adds MASK_VAL where k > q when used as lhsT against a [q, k]
        # tile; cmask2 is its transpose, for the [k, q]-layout pass-2 tiles.
        cmaskT = const.tile([128, 128], F16)
        nc.gpsimd.memset(cmaskT[:], MASK_VAL)
        nc.gpsimd.affine_select(
            out=cmaskT[:], in_=cmaskT[:],
            compare_op=mybir.AluOpType.is_gt, fill=0.0,
            base=0, pattern=[[-1, 128]], channel_multiplier=1)
        cmask2 = const.tile([128, 128], F16)
        cm_ps = ps_s.tile([128, 128], F16, tag="S", name="cmps")
        nc.tensor.transpose(cm_ps[:], cmaskT[:], ident[:])
        nc.vector.tensor_copy(cmask2[:], cm_ps[:])

        # ---- load inputs (transposed, fp16), one DMA per DIN-chunk piece
        xq = big.tile([128, 4, L], F16)
        xk = big.tile([128, 4, L], F16)
        xv = big.tile([128, 4, L], F16)
        wq = big.tile([128, 4, HD], F16)
        wk = big.tile([128, 4, HD], F16)
        wv = big.tile([128, 4, HD], F16)
        for t, d in ((wq, wq_d), (xq, qt_d), (wk, wk_d), (xk, kt_d),
                     (wv, wv_d), (xv, vt_d)):
            r = d.rearrange("(c p) l -> p c l", p=128)
            for c in range(4):
                nc.sync.dma_start(t[:, c, :], r[:, c, :])

        # QTE/KTE: per-head qT/kT in rows 0-63; rows 64-71 carry the -max
        # ride-along (QTE: per-q-tile negated maxes; KTE: constant one-hot).
        QTE = big.tile([128, 8, L], F16)
        KTE = big.tile([128, 8, L], F16)
        # spare rows zeroed so both passes can run full K=128 contraction
        # (zero rows add nothing); KTE rows 64-71 get the one-hot indicator
        # KTE[64+j, h, :] = (j == h) from the 8x8 identity block of `ident`.
        # All three fills go through broadcast DMAs to keep the DVE/Scalar
        # queues free during the load window.
        # v65: V rows with a ones column appended per head; the attention*V
        # matmul's 65th output column is then the softmax denominator.
        v65 = big.tile([128, 8, 8, 65], F16)
        nc.gpsimd.memset(v65[:, :, :, 64:65], 1.0)

        # PE warm-up: dummy matmuls while the loads stream in, so the
        # clock governor sees sustained activity before projections.
        warm = const.tile([128, 512], F16)
        nc.vector.memset(warm[:], 0.0)
        wps = ps_s.tile([128, 1024], F32, tag="S")
        for i in range(16):
            nc.tensor.matmul(wps[:, 0:512], lhsT=warm[:, 0:128], rhs=warm[:],
                             start=(i == 0), stop=(i == 15))

        zstrip = big.tile([128, L], F16)
        nc.vector.memset(zstrip[:], 0.0)
        ostrip = big.tile([128, L], F16)
        nc.gpsimd.memset(ostrip[:], 1.0)
        for j in range(8):
            nc.sync.dma_start(QTE[64:128, j, :], zstrip[64:128, :])
            nc.sync.dma_start(KTE[64:128, j, :], zstrip[64:128, :])
        for j in range(8):
            nc.sync.dma_start(KTE[64 + j:65 + j, j, :], ostrip[64 + j:65 + j, :])


        # ---- projections (fp16 matmuls, f32 psum)
        for w_sb, x_sb, dst in ((wq, xq, QTE), (wk, xk, KTE)):
            for t in range(4):
                for s in range(2):
                    pp = ps_s.tile([128, 512], F32, tag="S")
                    for c in range(4):
                        nc.tensor.matmul(
                            pp[:],
                            lhsT=w_sb[:, c, t * 128:(t + 1) * 128],
                            rhs=x_sb[:, c, s * 512:(s + 1) * 512],
                            start=(c == 0), stop=(c == 3))
                    sl = slice(s * 512, (s + 1) * 512)
                    nc.scalar.copy(dst[0:64, 2 * t, sl], pp[0:64, :])
                    nc.vector.tensor_copy(dst[0:64, 2 * t + 1, sl],
                                          pp[64:128, :])

        def emit_vproj():
            for lt in range(8):
                pp = ps_s.tile([128, 512], F32, name=f"ppv{lt}", tag="S")
                for c in range(4):
                    nc.tensor.matmul(
                        pp[:],
                        lhsT=xv[:, c, lt * 128:(lt + 1) * 128],
                        rhs=wv[:, c, :],
                        start=(c == 0), stop=(c == 3))
                dstv = v65[:, lt, :, 0:64]
                srcv = pp[:].rearrange("p (h d) -> p h d", h=8)
                if lt % 2 == 0:
                    nc.vector.tensor_copy(dstv, srcv)
                else:
                    nc.scalar.copy(dstv, srcv)

        # ---- attention, software-pipelined across q-tiles:
        # pass1(qt): per head: S = QK^T + mask (PE) -> row max (DVE, f16)
        # bridge(qt): nm [128,8] --PE transpose--> [8,128] -> broadcast into
        #             QTE rows 64-71 at this q-tile's columns
        # pass2(qt): per head: S'^T = K Q^T - m + mask (PE) -> exp (Scalar,
        #            writes P^T to SBUF) -> AV matmuls (65 cols)
        # Emission interleaves pass1(qt-1) with pass2(qt) so the PE never
        # waits on the per-q-tile max barrier.
        def emit_pass1(qt):
            Lq0, Lk = qt * 128, (qt + 1) * 128
            nm = sb.tile([128, 8], F16, name=f"nm{qt}", tag="nm", bufs=3)
            for h in range(8):
                S = ps_s.tile([128, 1024], F32, name=f"S{qt}_{h}", tag="S")
                for w in range(0, Lk, 512):
                    n = min(512, Lk - w)
                    diag = (w + n == Lk)
                    nc.tensor.matmul(
                        S[:, w:w + n],
                        lhsT=QTE[0:128, h, Lq0:Lq0 + 128],
                        rhs=KTE[0:128, h, w:w + n],
                        start=True, stop=not diag)
                    if diag:
                        nc.tensor.matmul(S[:, Lk - 128:Lk], lhsT=cmaskT[:],
                                         rhs=ident[:], start=False, stop=True)
                nc.vector.reduce_max(nm[:, h:h + 1], S[:, :Lk],
                                     axis=mybir.AxisListType.X, negate=True)
            return nm

        def emit_bridge(qt, nm):
            Lq0 = qt * 128
            nmT = ps_s.tile([8, 128], F16, name=f"nmT{qt}", tag="S")
            nc.tensor.transpose(nmT[:], nm[:], ident[:])
            nc.vector.tensor_copy(
                QTE[64:72, :, Lq0:Lq0 + 128],
                nmT[:].unsqueeze(1).broadcast_to([8, 8, 128]))

        def emit_st_exp(qt, h):
            Lq0, Lk, nkc = qt * 128, (qt + 1) * 128, qt + 1
            ST = ps_s.tile([128, 1024], F32, name=f"ST{qt}_{h}", tag="S")
            for kc in range(nkc):
                diag = (kc == qt)
                nc.tensor.matmul(
                    ST[:, kc * 128:(kc + 1) * 128],
                    lhsT=KTE[0:128, h, kc * 128:(kc + 1) * 128],
                    rhs=QTE[0:128, h, Lq0:Lq0 + 128],
                    start=True, stop=not diag)
                if diag:
                    nc.tensor.matmul(ST[:, kc * 128:(kc + 1) * 128],
                                     lhsT=cmask2[:], rhs=ident[:],
                                     start=False, stop=True)
            pTs = sb.tile([128, 8, 128], F16, name=f"pTs{qt}_{h}",
                          tag="pTs", bufs=4)
            pTf = pTs[:].rearrange("p c q -> p (c q)")
            nc.scalar.activation(pTf[:, :Lk], ST[:, :Lk],
                                 mybir.ActivationFunctionType.Exp)
            return pTs

        def emit_av(qt, h, pTs, avs):
            av = avs[h // 4]
            for kc in range(qt + 1):
                nc.tensor.matmul(
                    av[:, h % 4, :],
                    lhsT=pTs[:, kc, :],
                    rhs=v65[:, kc, h, :],
                    start=(kc == 0), stop=(kc == qt))

        def make_avsb(qt):
            return sb.tile([128, 8, 65], F32, name=f"avsb{qt}", tag="avsb",
                           bufs=2)

        def emit_finish(qt, avs, avsb):
            Lq0 = qt * 128
            nc.scalar.copy(avsb[:, 4:8, :], avs[1][:])
            rec = sb.tile([128, 8], F32, tag="rec")
            nc.vector.reciprocal(rec[:], avsb[:, :, 64])
            out_sb = sb.tile([128, HD], F32, tag="osb")
            nc.gpsimd.tensor_mul(
                out_sb[:].rearrange("p (h d) -> p h d", h=8),
                avsb[:, :, 0:64],
                rec[:].unsqueeze(2).broadcast_to([128, 8, 64]))
            nc.sync.dma_start(out_d[Lq0:Lq0 + 128, :], out_sb[:])

        def emit_s1_head(qt, h, nm):
            Lq0, Lk = qt * 128, (qt + 1) * 128
            S = ps_s.tile([128, 1024], F32, name=f"S{qt}_{h}", tag="S")
            for w in range(0, Lk, 512):
                n = min(512, Lk - w)
                diag = (w + n == Lk)
                nc.tensor.matmul(
                    S[:, w:w + n],
                    lhsT=QTE[0:128, h, Lq0:Lq0 + 128],
                    rhs=KTE[0:128, h, w:w + n],
                    start=True, stop=not diag)
                if diag:
                    nc.tensor.matmul(S[:, Lk - 128:Lk], lhsT=cmaskT[:],
                                     rhs=ident[:], start=False, stop=True)
            nc.vector.reduce_max(nm[:, h:h + 1], S[:, :Lk],
                                 axis=mybir.AxisListType.X, negate=True)

        nm = emit_pass1(7)
        emit_vproj()  # PE work covering the first q-tile's max latency
        emit_bridge(7, nm)
        # Software pipeline: inside iteration qt, heads 0-7 of pass 2 run
        # with AV lagging two heads behind exp; the next q-tile's pass 1
        # fills the first half; its bridge lands at h == 5; the last two AV
        # matmuls of qt are stitched between the first two STs of qt-1 so
        # the PE never waits on the trailing exps.
        carry = None  # (qt, pTss, avs, avsb) with AV h=6,7 + finish pending
        for qt in range(7, -1, -1):
            av_a = ps_av.tile([128, 4, 65], F32, name=f"ava{qt}", tag="av_a")
            av_b = ps_av.tile([128, 4, 65], F32, name=f"avb{qt}", tag="av_b")
            avs = (av_a, av_b)
            avsb = make_avsb(qt)
            if qt > 0:
                nm = sb.tile([128, 8], F16, name=f"nm{qt - 1}", tag="nm",
                             bufs=3)
            pTss = []
            for h in range(8):
                pTss.append(emit_st_exp(qt, h))
                if carry is not None and h < 2:
                    cqt, cpTss, cavs, cavsb = carry
                    emit_av(cqt, 6 + h, cpTss[6 + h], cavs)
                    if h == 1:
                        emit_finish(cqt, cavs, cavsb)
                        carry = None
                if qt > 0 and h < 4:
                    emit_s1_head(qt - 1, 2 * h, nm)
                    emit_s1_head(qt - 1, 2 * h + 1, nm)
                if qt > 0 and h == 5:
                    emit_bridge(qt - 1, nm)
                if h >= 2:
                    emit_av(qt, h - 2, pTss[h - 2], avs)
                if h == 7:
                    nc.scalar.copy(avsb[:, 0:4, :], av_a[:])
            carry = (qt, pTss, avs, avsb)
        cqt, cpTss, cavs, cavsb = carry
        emit_av(cqt, 6, cpTss[6], cavs)
        emit_av(cqt, 7, cpTss[7], cavs)
        emit_finish(cqt, cavs, cavsb)


def kernel(Q_seq, K_seq, V_seq, WQ, WK, WV, _trace=False):
    if "nc" not in _cached:
        _cached["nc"] = _build()
    nc = _cached["nc"]

    wq16 = (np.asarray(WQ, dtype=np.float32) * 0.125).astype(np.float16)
    wk16 = np.asarray(WK, dtype=np.float16)
    wv16 = np.asarray(WV, dtype=np.float16)
    in_maps = []
    for b in range(N_CORES):
        in_maps.append({
            "qt": np.ascontiguousarray(np.asarray(Q_seq[b]).T.astype(np.float16)),
            "kt": np.ascontiguousarray(np.asarray(K_seq[b]).T.astype(np.float16)),
            "vt": np.ascontiguousarray(np.asarray(V_seq[b]).T.astype(np.float16)),
            "wq": wq16, "wk": wk16, "wv": wv16,
        })
    res = run_bass_kernel_spmd(nc, in_maps, core_ids=list(range(N_CORES)),
                               trace=_trace)
    out = np.stack([res.results[b]["out"] for b in range(N_CORES)], axis=0)
    if _trace:
        kernel.last_exec_time_ns = res.exec_time_ns
        kernel.last_results = res
    return out
